# revision 24
# baseline (speedup 1.0000x reference)
"""Multi-head self-attention TRN2 kernel (data-parallel over batch).

Problem: B=8, S=1024, D=384, H=8, per-head full D->D projections,
causal + key-padding mask, softmax, out_linear (H*D)->D, query-mask output.

Sharding: batch b -> NeuronCore b (8 cores, no collectives).

Algebraic restructure (host precompute, exact):
  M_h = Wq_h @ Wk_h^T   ->  scores_raw = x M_h x^T   (K-projection eliminated)
  N_h = Wv_h @ Wo_h     ->  out = sum_h softmax(scores) @ (x N_h)  (out-proj eliminated)
  bias folds: Q.bk term is constant per query row -> cancels in softmax;
  bq.K term -> per-key exp bias column; (attn@bv)Wo = bv@Wo (softmax sums to 1)
  -> folded into bo on host. All biases are zero in this problem anyway.

Per-core dataflow (one batch element), transpose-free, all bf16 matmuls:
  xT [D,S] bf16 resident in SBUF (used 3 ways: P1/P2 moving, P3 stationary).
  For each head h:
    P1: Q'T[e,s] = M-chunks @ xT          (PE, psum [128,512], -> bf16 SBUF)
    P2: U[t,e]   = xT-chunks @ N          (PE, psum [128,384], -> bf16 SBUF)
    per 256-wide q group (4 groups, causally-live t-chunks only):
      P3: scoresT[t,s] psum = xT-chunk stationary @ Q'T   (raw, unscaled)
      diagonal chunks: min(scores, MTpat) in-psum (DVE), 2 const patterns
      attnT[t,s] = exp(scores*inv_sqrt_d + keybias[t]) -> bf16 SBUF (ACT)
      colsum[1,s] += ones^T @ attnT       (PE, M=1 matmul, per 512-half)
    per 512-half: colsum -> DRAM bounce -> [128,4] -> +eps -> recip -> *maskq
    P4 per q-tile (128): psum[s,e] = sum_t attnT-chunk stationary @ U
      out_acc[s,:] += psum * recip'[s]    (DVE STT)
  out = out_acc (maskq,bo pre-folded) -> DRAM
"""

import os
from contextlib import ExitStack

import numpy as np

B, S, D, H = 8, 1024, 384, 8
P = 128
DC = D // P          # 3 partition chunks of the d/e axes
NQT = S // P         # 8 q/t tiles of 128
GW = 256             # q-group width for scores/exp
NG = S // GW         # 4 groups
BIG = 3.0e38
INV_SQRT_D = float(1.0 / np.sqrt(np.float32(D), dtype=np.float32))
KNEG = -120.0                      # exp bias for masked keys -> exp==0 in bf16
RAWNEG = float(KNEG / INV_SQRT_D)  # raw-score causal fill; scaled -> -120

CFG = {
    "dt": os.environ.get("MHA_DT", "bf16"),
    # P3 (scores) matmul: "bf16" | "fp8" (fp8 = DoubleRow, x-side hi/lo pair,
    # Q'-store e4m3; ~2x faster scores at ~1.7e-2 relerr vs 3e-3)
    "p3": os.environ.get("MHA_P3", "bf16"),
}

_BUILT = None  # (nc, cfg)


def _dt(kind):
    import concourse.mybir as mybir

    return {"bf16": mybir.dt.bfloat16, "f32r": mybir.dt.float32r,
            "f32": mybir.dt.float32}[kind]


def _np_dt(kind):
    import ml_dtypes

    return ml_dtypes.bfloat16 if kind == "bf16" else np.float32


def build(cfg=None):
    import concourse.bass as bass
    import concourse.bacc as bacc
    import concourse.tile as tile
    import concourse.mybir as mybir

    cfg = dict(CFG if cfg is None else cfg)
    f32 = mybir.dt.float32
    u32 = mybir.dt.uint32
    dt = _dt(cfg["dt"])
    p3_fp8 = cfg["p3"] == "fp8"
    f8 = mybir.dt.float8e4

    nc = bacc.Bacc("TRN2", target_bir_lowering=False, debug=False)

    xT_d = nc.dram_tensor("xT", [D, S], dt, kind="ExternalInput")
    if p3_fp8:
        xp_d = nc.dram_tensor("xp8", [D, 2, S], f8, kind="ExternalInput")
    m_d = nc.dram_tensor("M", [H, D, D], dt, kind="ExternalInput")
    n_d = nc.dram_tensor("N", [H, D, D], dt, kind="ExternalInput")
    kb_d = nc.dram_tensor("kbT", [P, NQT], f32, kind="ExternalInput")
    maskq_d = nc.dram_tensor("maskq", [S], f32, kind="ExternalInput")
    bo_d = nc.dram_tensor("bo", [P, D], f32, kind="ExternalInput")
    out_d = nc.dram_tensor("out", [S, D], f32, kind="ExternalOutput")
    # per-(head, group) bounce rows for column sums
    scr_d = nc.dram_tensor("sum_scratch", [H * NG, GW], f32)

    with tile.TileContext(nc) as tc, ExitStack() as ctx:
        consts = ctx.enter_context(tc.tile_pool(name="consts", bufs=1))
        wpool = ctx.enter_context(tc.tile_pool(name="wpool", bufs=2))
        qpool = ctx.enter_context(tc.tile_pool(name="qpool", bufs=2))
        upool = ctx.enter_context(tc.tile_pool(name="upool", bufs=2))
        apool = ctx.enter_context(tc.tile_pool(name="apool", bufs=3))
        small = ctx.enter_context(tc.tile_pool(name="small", bufs=8))
        ps_pj = ctx.enter_context(tc.tile_pool(name="ps_pj", bufs=2, space="PSUM"))
        ps_sc = ctx.enter_context(tc.tile_pool(name="ps_sc", bufs=2, space="PSUM"))
        ps_pv = ctx.enter_context(tc.tile_pool(name="ps_pv", bufs=3, space="PSUM"))
        ps_cs = ctx.enter_context(tc.tile_pool(name="ps_cs", bufs=1, space="PSUM"))

        # ---- setup ----
        # weights for head 0 and xT first so P1(h=0) starts ASAP (it also
        # serves as the PE clock-ramp warmup)
        wtiles = {}

        def _fetch_w(h):
            m_sb = wpool.tile([P, DC, D], dt, tag="m")
            n_sb = wpool.tile([P, DC, D], dt, tag="n")
            nc.sync.dma_start(
                out=m_sb, in_=m_d.ap()[h].rearrange("(c p) e -> p c e", p=P)
            )
            nc.sync.dma_start(
                out=n_sb, in_=n_d.ap()[h].rearrange("(c p) e -> p c e", p=P)
            )
            wtiles[h] = (m_sb, n_sb)

        _fetch_w(0)
        # xT as two s-half tiles so P1 can start after the first half lands
        xTh = []
        for sh in range(2):
            t_ = consts.tile([P, DC, 512], dt, tag=f"xT{sh}")
            nc.sync.dma_start(
                out=t_,
                in_=xT_d.ap()[:, sh * 512 : (sh + 1) * 512].rearrange(
                    "(c p) s -> p c s", p=P
                ),
            )
            xTh.append(t_)
        _fetch_w(1)
        if p3_fp8:
            xp_sb = consts.tile([P, DC, 2, S], f8, tag="xp8")
            nc.sync.dma_start(
                out=xp_sb,
                in_=xp_d.ap().rearrange("(c p) two s -> p c two s", p=P),
            )

        kb_sb = consts.tile([P, NQT], f32, tag="kbT")
        nc.sync.dma_start(out=kb_sb, in_=kb_d.ap())

        maskq_sb = consts.tile([P, NQT], f32, tag="maskq")
        nc.sync.dma_start(
            out=maskq_sb, in_=maskq_d.ap().rearrange("(q p) -> p q", p=P)
        )

        bo_sb = consts.tile([P, D], f32, tag="bo")
        nc.sync.dma_start(out=bo_sb, in_=bo_d.ap())

        ones_sb = consts.tile([P, 1], dt, tag="ones")
        nc.vector.memset(ones_sb, 1.0)

        # causal min-mask patterns for diagonal chunks: [P, GW] f32,
        # keep (BIG) where s_local >= t_local + off, else RAWNEG.
        mt = []
        for off in (0, 128):
            t_ = consts.tile([P, GW], f32, tag=f"mt{off}")
            nc.vector.memset(t_, BIG)
            nc.gpsimd.affine_select(
                out=t_, in_=t_,
                compare_op=mybir.AluOpType.is_ge,
                fill=RAWNEG, base=-off, channel_multiplier=-1,
                pattern=[[1, GW]],
            )
            mt.append(t_)

        # out accumulator, init = bo * maskq (bo has maskq-invariant fold done host-side)
        out_acc = consts.tile([P, NQT, D], f32, tag="out_acc")
        for qt in range(NQT):
            nc.vector.tensor_scalar_mul(
                out=out_acc[:, qt, :], in0=bo_sb, scalar1=maskq_sb[:, qt : qt + 1]
            )

        # ---- per-head pipeline ----
        n_heads = int(os.environ.get("MHA_HEADS", str(H)))

        for h in range(n_heads):
            m_sb, n_sb = wtiles.pop(h)

            # P1: Q'T [e, s]  (fp8 store when P3 runs in DoubleRow mode)
            qp_sb = qpool.tile([P, DC, S], f8 if p3_fp8 else dt, tag="QT")
            for sh in range(S // 512):
                for ec in range(DC):
                    ps = ps_pj.tile([P, 512], f32, tag="pj")
                    for dc in range(DC):
                        nc.tensor.matmul(
                            ps,
                            m_sb[:, dc, ec * P : (ec + 1) * P],
                            xTh[sh][:, dc, :],
                            start=(dc == 0),
                            stop=(dc == DC - 1),
                        )
                    nc.scalar.copy(
                        out=qp_sb[:, ec, sh * 512 : (sh + 1) * 512], in_=ps
                    )

            # P2: U [t, e]
            u_sb = upool.tile([P, NQT, D], dt, tag="U")
            for tt in range(NQT):
                psu = ps_pv.tile([P, D], f32, tag="pv", name="ps_u")
                for dc in range(DC):
                    nc.tensor.matmul(
                        psu,
                        xTh[tt // 4][:, dc, (tt % 4) * P : (tt % 4 + 1) * P],
                        n_sb[:, dc, :],
                        start=(dc == 0),
                        stop=(dc == DC - 1),
                    )
                nc.vector.tensor_copy(out=u_sb[:, tt, :], in_=psu)

            # prefetch weights two heads ahead (m/n last read in P1/P2 above)
            if h + 2 < n_heads:
                _fetch_w(h + 2)

            # attention per 256-wide s-group
            for qg in range(NG):
                ntt = 2 * qg + 2          # live t-chunks for this group
                s0 = qg * GW
                att_t = apool.tile([P, NQT, GW], dt, tag="attnT", name="att_t")
                ps_sums = ps_cs.tile([1, GW], f32, tag="cs")
                for tt in range(ntt):
                    ps_s = ps_sc.tile([P, GW], f32, tag="sc")
                    for ec in range(DC):
                        if p3_fp8:
                            nc.tensor.matmul(
                                ps_s,
                                xp_sb[:, ec, :, tt * P : (tt + 1) * P],
                                qp_sb[:, ec, s0 : s0 + GW]
                                .unsqueeze(1)
                                .broadcast_to([P, 2, GW]),
                                start=(ec == 0),
                                stop=(ec == DC - 1),
                                perf_mode=mybir.MatmulPerfMode.DoubleRow,
                            )
                        else:
                            nc.tensor.matmul(
                                ps_s,
                                xTh[tt // 4][
                                    :, ec, (tt % 4) * P : (tt % 4 + 1) * P
                                ],
                                qp_sb[:, ec, s0 : s0 + GW],
                                start=(ec == 0),
                                stop=(ec == DC - 1),
                            )
                    if tt >= 2 * qg:  # diagonal chunk: causal min pre-exp
                        nc.vector.tensor_tensor(
                            out=ps_s, in0=ps_s, in1=mt[tt - 2 * qg],
                            op=mybir.AluOpType.min,
                        )
                    nc.scalar.activation(
                        out=att_t[:, tt, :],
                        in_=ps_s,
                        func=mybir.ActivationFunctionType.Exp,
                        scale=INV_SQRT_D,
                        bias=kb_sb[:, tt : tt + 1],
                    )
                for tt in range(ntt):
                    nc.tensor.matmul(
                        ps_sums,
                        ones_sb,
                        att_t[:, tt, :],
                        start=(tt == 0),
                        stop=(tt == ntt - 1),
                    )

                # bounce this group's colsums, recip, fold maskq
                srow = small.tile([1, GW], f32, tag="srow")
                nc.vector.tensor_copy(out=srow, in_=ps_sums)
                scr = scr_d.ap()[h * NG + qg]
                nc.sync.dma_start(out=scr, in_=srow)
                scat = small.tile([P, 2], f32, tag="scat")
                nc.sync.dma_start(
                    out=scat,
                    in_=bass.AP(
                        tensor=scr.tensor, offset=scr.offset, ap=[[1, P], [P, 2]]
                    ),
                )
                guard = small.tile([P, 2], f32, tag="guard")
                nc.vector.tensor_scalar_add(out=guard, in0=scat, scalar1=1e-30)
                recip = small.tile([P, 2], f32, tag="recip")
                nc.vector.reciprocal(out=recip, in_=guard)
                recipm = small.tile([P, 2], f32, tag="recipm")
                nc.vector.tensor_tensor(
                    out=recipm, in0=recip,
                    in1=maskq_sb[:, 2 * qg : 2 * qg + 2],
                    op=mybir.AluOpType.mult,
                )

                # P4 for this group's 2 q-tiles (128-granular causality)
                for qi in range(2):
                    qt = 2 * qg + qi
                    ps_p = ps_pv.tile([P, D], f32, tag="pv")
                    for tt in range(qt + 1):
                        nc.tensor.matmul(
                            ps_p,
                            att_t[:, tt, qi * P : (qi + 1) * P],
                            u_sb[:, tt, :],
                            start=(tt == 0),
                            stop=(tt == qt),
                        )
                    nc.vector.scalar_tensor_tensor(
                        out=out_acc[:, qt, :],
                        in0=ps_p,
                        scalar=recipm[:, qi : qi + 1],
                        in1=out_acc[:, qt, :],
                        op0=mybir.AluOpType.mult,
                        op1=mybir.AluOpType.add,
                    )

        # ---- final store (maskq and bo already folded into out_acc) ----
        for qt in range(NQT):
            nc.sync.dma_start(
                out=out_d.ap()[qt * P : (qt + 1) * P, :], in_=out_acc[:, qt, :]
            )

    nc.compile()
    return nc


def _in_maps(x, mask, Wq, bq, Wk, bk, Wv, bv, Wo, bo, cfg):
    np_dt = _np_dt(cfg["dt"])
    f32 = np.float32
    x = np.asarray(x, f32)
    Wq = np.asarray(Wq, f32)
    Wk = np.asarray(Wk, f32)
    Wv = np.asarray(Wv, f32)
    Wo = np.asarray(Wo, f32).reshape(H, D, D)
    bq = np.asarray(bq, f32)
    bk = np.asarray(bk, f32)
    bv = np.asarray(bv, f32)
    bo = np.asarray(bo, f32)

    # host precompute: M = Wq Wk^T, N = Wv Wo  (fp32)
    M = np.einsum("hde,hfe->hdf", Wq, Wk)
    N = np.einsum("hde,hef->hdf", Wv, Wo)

    # bias folds (all-zero biases in this problem, kept for generality):
    #   scores += bq.K_t (per-key) -> raw bias columns; Q.bk const/row -> cancels
    #   out += sum_h (bv_h @ Wo_h) + bo  (attn rows sum to 1)
    bo_f = bo + np.einsum("hd,hdf->f", bv, Wo)

    m = np.asarray(mask) != 0
    maskq = m.astype(f32)

    shared = {
        "M": M.astype(np_dt),
        "N": N.astype(np_dt),
        "bo": np.broadcast_to(bo_f[None, :], (P, D)).copy(),
    }
    xT = np.ascontiguousarray(x.transpose(0, 2, 1))  # [B, D, S]
    maps = []
    for b in range(B):
        # per-key exp bias: 0 valid / KNEG masked; plus bq.K_t fold (zero here)
        kb = np.where(m[b], 0.0, np.float32(KNEG)).astype(f32)
        mp = {
            "xT": xT[b].astype(np_dt),
            "kbT": np.ascontiguousarray(kb.reshape(NQT, P).T),
            "maskq": maskq[b],
            **shared,
        }
        if cfg["p3"] == "fp8":
            import ml_dtypes

            E4 = ml_dtypes.float8_e4m3
            hi = xT[b].astype(E4)
            lo = (xT[b] - hi.astype(f32)).astype(E4)
            mp["xp8"] = np.ascontiguousarray(
                np.stack([hi, lo], axis=1)  # [D, 2, S]
            )
        maps.append(mp)
    return maps


def run(inputs, trace=False, cfg=None):
    """inputs: dict from setup_inputs(). Returns (out [B,S,D] f32, results)."""
    from concourse.bass_utils import run_bass_kernel_spmd

    global _BUILT
    cfg = dict(CFG if cfg is None else cfg)
    if _BUILT is None or _BUILT[1] != cfg:
        _BUILT = (build(cfg), cfg)
    nc = _BUILT[0]
    in_maps = _in_maps(**inputs, cfg=cfg)
    res = run_bass_kernel_spmd(
        nc, in_maps, core_ids=list(range(B)), trace=trace
    )
    out = np.stack([np.asarray(res.results[b]["out"], np.float32) for b in range(B)])
    return out, res


def kernel(**inputs):
    out, _ = run(inputs, trace=False)
    return out


# revision 25
# speedup vs baseline: 1.2084x; 1.2084x over previous
"""Multi-head self-attention TRN2 kernel (data-parallel over batch).

Problem: B=8, S=1024, D=384, H=8, per-head full D->D projections,
causal + key-padding mask, softmax, out_linear (H*D)->D, query-mask output.

Sharding: batch b -> NeuronCore b (8 cores, no collectives).

Algebraic restructure (host precompute, exact):
  M_h = Wq_h @ Wk_h^T   ->  scores_raw = x M_h x^T   (K-projection eliminated)
  N_h = Wv_h @ Wo_h     ->  out = sum_h softmax(scores) @ (x N_h)  (out-proj eliminated)
  bias folds: Q.bk term is constant per query row -> cancels in softmax;
  bq.K term -> per-key exp bias column; (attn@bv)Wo = bv@Wo (softmax rows sum
  to 1) -> folded into bo on host. All biases are zero in this problem anyway.

Per-core dataflow (one batch element), transpose-free, all bf16 matmuls:
  xT [D,S] bf16 resident in SBUF, split [sh][dc] for DMA/compute pipelining.
  For each head h:
    P1: Q'T[e,s] = M-chunks @ xT          (PE, psum [128,512], -> bf16 SBUF)
    P2: U[t,e|1] = xT-chunks @ N, col 384 = ones  (PE, -> bf16 SBUF)
    per 256-wide q group (4 groups, causally-live t-chunks only):
      P3: scoresT[t,s] psum = xT-chunk stationary @ Q'T   (raw, unscaled)
      diagonal chunks: min(scores, MTpat) in-psum (DVE), 2 const patterns
      attnT[t,s] = exp(scores*inv_sqrt_d + keybias[t]) -> bf16 SBUF (ACT)
      P4 per q-tile (128): psum[s, 0:385] = sum_t attnT-chunk stat @ U
        -> psum[:,384] is the colsum; recip'[s] = maskq/(colsum+eps) (DVE)
        -> out_acc[s,:] += psum[:, :384] * recip'[s]    (DVE STT)
  out = out_acc (maskq,bo pre-folded) -> DRAM
"""

import os
from contextlib import ExitStack

import numpy as np

B, S, D, H = 8, 1024, 384, 8
P = 128
DC = D // P          # 3 partition chunks of the d/e axes
NQT = S // P         # 8 q/t tiles of 128
GW = 256             # q-group width for scores/exp
NG = S // GW         # 4 groups
DU = D + 1           # U width incl. ones column for in-P4 colsum
BIG = 3.0e38
INV_SQRT_D = float(1.0 / np.sqrt(np.float32(D), dtype=np.float32))
KNEG = -120.0                      # exp bias for masked keys -> exp==0 in bf16
RAWNEG = float(KNEG / INV_SQRT_D)  # raw-score causal fill; scaled -> -120

CFG = {"dt": os.environ.get("MHA_DT", "bf16")}

_BUILT = None  # (nc, cfg)


def _dt(kind):
    import concourse.mybir as mybir

    return {"bf16": mybir.dt.bfloat16, "f32r": mybir.dt.float32r,
            "f32": mybir.dt.float32}[kind]


def _np_dt(kind):
    import ml_dtypes

    return ml_dtypes.bfloat16 if kind == "bf16" else np.float32


def build(cfg=None):
    import concourse.bass as bass
    import concourse.bacc as bacc
    import concourse.tile as tile
    import concourse.mybir as mybir

    cfg = dict(CFG if cfg is None else cfg)
    f32 = mybir.dt.float32
    dt = _dt(cfg["dt"])

    nc = bacc.Bacc("TRN2", target_bir_lowering=False, debug=False)

    xT_d = nc.dram_tensor("xT", [D, S], dt, kind="ExternalInput")
    m_d = nc.dram_tensor("M", [H, D, D], dt, kind="ExternalInput")
    n_d = nc.dram_tensor("N", [H, D, D], dt, kind="ExternalInput")
    kb_d = nc.dram_tensor("kbT", [P, NQT], f32, kind="ExternalInput")
    maskq_d = nc.dram_tensor("maskq", [S], f32, kind="ExternalInput")
    bo_d = nc.dram_tensor("bo", [P, D], f32, kind="ExternalInput")
    out_d = nc.dram_tensor("out", [S, D], f32, kind="ExternalOutput")

    with tile.TileContext(nc) as tc, ExitStack() as ctx:
        consts = ctx.enter_context(tc.tile_pool(name="consts", bufs=1))
        wpool = ctx.enter_context(tc.tile_pool(name="wpool", bufs=2))
        qpool = ctx.enter_context(tc.tile_pool(name="qpool", bufs=2))
        upool = ctx.enter_context(tc.tile_pool(name="upool", bufs=2))
        apool = ctx.enter_context(tc.tile_pool(name="apool", bufs=3))
        small = ctx.enter_context(tc.tile_pool(name="small", bufs=8))
        ps_pj = ctx.enter_context(tc.tile_pool(name="ps_pj", bufs=2, space="PSUM"))
        ps_sc = ctx.enter_context(tc.tile_pool(name="ps_sc", bufs=3, space="PSUM"))
        ps_pv = ctx.enter_context(tc.tile_pool(name="ps_pv", bufs=3, space="PSUM"))

        # ---- setup: head-0 weights and xT first so P1(h=0) starts ASAP
        # (it doubles as the PE clock-ramp warmup). Everything split by dc
        # chunk so PE work starts while later DMAs stream in.
        wtiles = {}

        def _fetch_w(h):
            ms, ns = [], []
            for dc in range(DC):
                t_ = wpool.tile([P, D], dt, tag=f"m{dc}")
                nc.sync.dma_start(
                    out=t_, in_=m_d.ap()[h, dc * P : (dc + 1) * P, :]
                )
                ms.append(t_)
            for dc in range(DC):
                t_ = wpool.tile([P, D], dt, tag=f"n{dc}")
                nc.sync.dma_start(
                    out=t_, in_=n_d.ap()[h, dc * P : (dc + 1) * P, :]
                )
                ns.append(t_)
            wtiles[h] = (ms, ns)

        xts = [[None] * DC for _ in range(2)]

        def _fetch_x(sh):
            for dc in range(DC):
                t_ = consts.tile([P, 512], dt, tag=f"xT{sh}{dc}")
                nc.sync.dma_start(
                    out=t_,
                    in_=xT_d.ap()[
                        dc * P : (dc + 1) * P, sh * 512 : (sh + 1) * 512
                    ],
                )
                xts[sh][dc] = t_

        # interleave: m(h0) dc0, x(0) dc0 needed first
        _fetch_w(0)
        _fetch_x(0)
        _fetch_x(1)
        _fetch_w(1)

        kb_sb = consts.tile([P, NQT], f32, tag="kbT")
        nc.sync.dma_start(out=kb_sb, in_=kb_d.ap())

        maskq_sb = consts.tile([P, NQT], f32, tag="maskq")
        nc.sync.dma_start(
            out=maskq_sb, in_=maskq_d.ap().rearrange("(q p) -> p q", p=P)
        )

        bo_sb = consts.tile([P, D], f32, tag="bo")
        nc.sync.dma_start(out=bo_sb, in_=bo_d.ap())

        # causal min-mask patterns for diagonal chunks: [P, GW] f32,
        # keep (BIG) where s_local >= t_local + off, else RAWNEG.
        mt = []
        for off in (0, 128):
            t_ = consts.tile([P, GW], f32, tag=f"mt{off}")
            nc.vector.memset(t_, BIG)
            nc.gpsimd.affine_select(
                out=t_, in_=t_,
                compare_op=mybir.AluOpType.is_ge,
                fill=RAWNEG, base=-off, channel_multiplier=-1,
                pattern=[[1, GW]],
            )
            mt.append(t_)

        # out accumulator, init = bo * maskq (bias folds done host-side)
        out_acc = consts.tile([P, NQT, D], f32, tag="out_acc")
        for qt in range(NQT):
            nc.vector.tensor_scalar_mul(
                out=out_acc[:, qt, :], in0=bo_sb, scalar1=maskq_sb[:, qt : qt + 1]
            )

        # ---- per-head pipeline ----
        n_heads = int(os.environ.get("MHA_HEADS", str(H)))
        for h in range(n_heads):
            m_t, n_t = wtiles.pop(h)

            # P1: Q'T [e, s]
            qp_sb = qpool.tile([P, DC, S], dt, tag="QT")
            for sh in range(S // 512):
                for ec in range(DC):
                    ps = ps_pj.tile([P, 512], f32, tag="pj")
                    for dc in range(DC):
                        nc.tensor.matmul(
                            ps,
                            m_t[dc][:, ec * P : (ec + 1) * P],
                            xts[sh][dc],
                            start=(dc == 0),
                            stop=(dc == DC - 1),
                        )
                    nc.scalar.copy(
                        out=qp_sb[:, ec, sh * 512 : (sh + 1) * 512], in_=ps
                    )

            # P2: U [t, e] + ones column 384
            u_sb = upool.tile([P, NQT, DU], dt, tag="U")
            nc.vector.memset(u_sb[:, :, D], 1.0)
            for tt in range(NQT):
                psu = ps_pv.tile([P, DU], f32, tag="pv", name="ps_u")
                for dc in range(DC):
                    nc.tensor.matmul(
                        psu[:, :D],
                        xts[tt // 4][dc][:, (tt % 4) * P : (tt % 4 + 1) * P],
                        n_t[dc],
                        start=(dc == 0),
                        stop=(dc == DC - 1),
                    )
                nc.vector.tensor_copy(out=u_sb[:, tt, :D], in_=psu[:, :D])

            # prefetch weights two heads ahead (m/n last read in P1/P2 above)
            if h + 2 < n_heads:
                _fetch_w(h + 2)

            # attention per 256-wide s-group
            for qg in range(NG):
                ntt = 2 * qg + 2          # live t-chunks for this group
                s0 = qg * GW
                att_t = apool.tile([P, NQT, GW], dt, tag="attnT", name="att_t")
                for tt in range(ntt):
                    ps_s = ps_sc.tile([P, GW], f32, tag="sc")
                    for ec in range(DC):
                        nc.tensor.matmul(
                            ps_s,
                            xts[tt // 4][ec][:, (tt % 4) * P : (tt % 4 + 1) * P],
                            qp_sb[:, ec, s0 : s0 + GW],
                            start=(ec == 0),
                            stop=(ec == DC - 1),
                        )
                    if tt >= 2 * qg:  # diagonal chunk: causal min pre-exp
                        nc.vector.tensor_tensor(
                            out=ps_s, in0=ps_s, in1=mt[tt - 2 * qg],
                            op=mybir.AluOpType.min,
                        )
                    nc.scalar.activation(
                        out=att_t[:, tt, :],
                        in_=ps_s,
                        func=mybir.ActivationFunctionType.Exp,
                        scale=INV_SQRT_D,
                        bias=kb_sb[:, tt : tt + 1],
                    )

                # P4 for this group's 2 q-tiles; colsum rides in column 384
                for qi in range(2):
                    qt = 2 * qg + qi
                    ps_p = ps_pv.tile([P, DU], f32, tag="pv")
                    for tt in range(qt + 1):
                        nc.tensor.matmul(
                            ps_p,
                            att_t[:, tt, qi * P : (qi + 1) * P],
                            u_sb[:, tt, :],
                            start=(tt == 0),
                            stop=(tt == qt),
                        )
                    guard = small.tile([P, 1], f32, tag="guard")
                    nc.vector.tensor_scalar_add(
                        out=guard, in0=ps_p[:, D : D + 1], scalar1=1e-30
                    )
                    recip = small.tile([P, 1], f32, tag="recip")
                    nc.vector.reciprocal(out=recip, in_=guard)
                    recipm = small.tile([P, 1], f32, tag="recipm")
                    nc.vector.tensor_tensor(
                        out=recipm, in0=recip,
                        in1=maskq_sb[:, qt : qt + 1],
                        op=mybir.AluOpType.mult,
                    )
                    nc.vector.scalar_tensor_tensor(
                        out=out_acc[:, qt, :],
                        in0=ps_p[:, :D],
                        scalar=recipm,
                        in1=out_acc[:, qt, :],
                        op0=mybir.AluOpType.mult,
                        op1=mybir.AluOpType.add,
                    )

        # ---- final store (maskq and bo already folded into out_acc) ----
        for qt in range(NQT):
            nc.sync.dma_start(
                out=out_d.ap()[qt * P : (qt + 1) * P, :], in_=out_acc[:, qt, :]
            )

    nc.compile()
    return nc


def _in_maps(x, mask, Wq, bq, Wk, bk, Wv, bv, Wo, bo, cfg):
    np_dt = _np_dt(cfg["dt"])
    f32 = np.float32
    x = np.asarray(x, f32)
    Wq = np.asarray(Wq, f32)
    Wk = np.asarray(Wk, f32)
    Wv = np.asarray(Wv, f32)
    Wo = np.asarray(Wo, f32).reshape(H, D, D)
    bq = np.asarray(bq, f32)
    bk = np.asarray(bk, f32)
    bv = np.asarray(bv, f32)
    bo = np.asarray(bo, f32)

    # host precompute: M = Wq Wk^T, N = Wv Wo  (fp32)
    M = np.einsum("hde,hfe->hdf", Wq, Wk)
    N = np.einsum("hde,hef->hdf", Wv, Wo)

    # bias folds (all-zero biases in this problem, kept for generality):
    #   scores += bq.K_t (per-key) -> raw bias columns; Q.bk const/row -> cancels
    #   out += sum_h (bv_h @ Wo_h) + bo  (attn rows sum to 1)
    bo_f = bo + np.einsum("hd,hdf->f", bv, Wo)

    m = np.asarray(mask) != 0
    maskq = m.astype(f32)

    shared = {
        "M": M.astype(np_dt),
        "N": N.astype(np_dt),
        "bo": np.broadcast_to(bo_f[None, :], (P, D)).copy(),
    }
    xT = np.ascontiguousarray(x.transpose(0, 2, 1))  # [B, D, S]
    maps = []
    for b in range(B):
        # per-key exp bias: 0 valid / KNEG masked; plus bq.K_t fold (zero here)
        kb = np.where(m[b], 0.0, np.float32(KNEG)).astype(f32)
        maps.append(
            {
                "xT": xT[b].astype(np_dt),
                "kbT": np.ascontiguousarray(kb.reshape(NQT, P).T),
                "maskq": maskq[b],
                **shared,
            }
        )
    return maps


def run(inputs, trace=False, cfg=None):
    """inputs: dict from setup_inputs(). Returns (out [B,S,D] f32, results)."""
    from concourse.bass_utils import run_bass_kernel_spmd

    global _BUILT
    cfg = dict(CFG if cfg is None else cfg)
    if _BUILT is None or _BUILT[1] != cfg:
        _BUILT = (build(cfg), cfg)
    nc = _BUILT[0]
    in_maps = _in_maps(**inputs, cfg=cfg)
    res = run_bass_kernel_spmd(
        nc, in_maps, core_ids=list(range(B)), trace=trace
    )
    out = np.stack([np.asarray(res.results[b]["out"], np.float32) for b in range(B)])
    return out, res


def kernel(**inputs):
    out, _ = run(inputs, trace=False)
    return out


# revision 27
# speedup vs baseline: 1.2117x; 1.0027x over previous
"""Multi-head self-attention TRN2 kernel (data-parallel over batch).

Problem: B=8, S=1024, D=384, H=8, per-head full D->D projections,
causal + key-padding mask, softmax, out_linear (H*D)->D, query-mask output.

Sharding: batch b -> NeuronCore b (8 cores, no collectives).

Algebraic restructure (host precompute, exact):
  M_h = Wq_h @ Wk_h^T   ->  scores_raw = x M_h x^T   (K-projection eliminated)
  N_h = Wv_h @ Wo_h     ->  out = sum_h softmax(scores) @ (x N_h)  (out-proj eliminated)
  bias folds: Q.bk term is constant per query row -> cancels in softmax;
  bq.K term -> per-key exp bias column; (attn@bv)Wo = bv@Wo (softmax rows sum
  to 1) -> folded into bo on host. All biases are zero in this problem anyway.

Per-core dataflow (one batch element), transpose-free, all bf16 matmuls:
  xT [D,S] bf16 resident in SBUF, split [sh][dc] for DMA/compute pipelining.
  For each head h:
    P1: Q'T[e,s] = M-chunks @ xT          (PE, psum [128,512], -> bf16 SBUF)
    P2: U[t,e|1] = xT-chunks @ N, col 384 = ones  (PE, -> bf16 SBUF)
    per 256-wide q group (4 groups, causally-live t-chunks only):
      P3: scoresT[t,s] psum = xT-chunk stationary @ Q'T   (raw, unscaled)
      diagonal chunks: min(scores, MTpat) in-psum (DVE), 2 const patterns
      attnT[t,s] = exp(scores*inv_sqrt_d + keybias[t]) -> bf16 SBUF (ACT)
      P4 per q-tile (128): psum[s, 0:385] = sum_t attnT-chunk stat @ U
        -> psum[:,384] is the colsum; recip'[s] = maskq/(colsum+eps) (DVE)
        -> out_acc[s,:] += psum[:, :384] * recip'[s]    (DVE STT)
  out = out_acc (maskq,bo pre-folded) -> DRAM
"""

import os
from contextlib import ExitStack

import numpy as np

B, S, D, H = 8, 1024, 384, 8
P = 128
DC = D // P          # 3 partition chunks of the d/e axes
NQT = S // P         # 8 q/t tiles of 128
GW = 256             # q-group width for scores/exp
NG = S // GW         # 4 groups
DU = D + 1           # U width incl. ones column for in-P4 colsum
BIG = 3.0e38
INV_SQRT_D = float(1.0 / np.sqrt(np.float32(D), dtype=np.float32))
KNEG = -120.0                      # exp bias for masked keys -> exp==0 in bf16
RAWNEG = float(KNEG / INV_SQRT_D)  # raw-score causal fill; scaled -> -120

CFG = {"dt": os.environ.get("MHA_DT", "bf16")}

_BUILT = None  # (nc, cfg)


def _dt(kind):
    import concourse.mybir as mybir

    return {"bf16": mybir.dt.bfloat16, "f32r": mybir.dt.float32r,
            "f32": mybir.dt.float32}[kind]


def _np_dt(kind):
    import ml_dtypes

    return ml_dtypes.bfloat16 if kind == "bf16" else np.float32


def build(cfg=None):
    import concourse.bass as bass
    import concourse.bacc as bacc
    import concourse.tile as tile
    import concourse.mybir as mybir

    cfg = dict(CFG if cfg is None else cfg)
    f32 = mybir.dt.float32
    dt = _dt(cfg["dt"])

    nc = bacc.Bacc("TRN2", target_bir_lowering=False, debug=False)

    xT_d = nc.dram_tensor("xT", [D, S], dt, kind="ExternalInput")
    m_d = nc.dram_tensor("M", [H, D, D], dt, kind="ExternalInput")
    n_d = nc.dram_tensor("N", [H, D, D], dt, kind="ExternalInput")
    kb_d = nc.dram_tensor("kbT", [P, NQT], f32, kind="ExternalInput")
    maskq_d = nc.dram_tensor("maskq", [S], f32, kind="ExternalInput")
    bo_d = nc.dram_tensor("bo", [P, D], f32, kind="ExternalInput")
    out_d = nc.dram_tensor("out", [S, D], f32, kind="ExternalOutput")

    with tile.TileContext(nc) as tc, ExitStack() as ctx:
        consts = ctx.enter_context(tc.tile_pool(name="consts", bufs=1))
        wpool = ctx.enter_context(tc.tile_pool(name="wpool", bufs=2))
        qpool = ctx.enter_context(tc.tile_pool(name="qpool", bufs=2))
        upool = ctx.enter_context(tc.tile_pool(name="upool", bufs=2))
        apool = ctx.enter_context(tc.tile_pool(name="apool", bufs=3))
        small = ctx.enter_context(tc.tile_pool(name="small", bufs=8))
        ps_pj = ctx.enter_context(tc.tile_pool(name="ps_pj", bufs=2, space="PSUM"))
        ps_sc = ctx.enter_context(tc.tile_pool(name="ps_sc", bufs=3, space="PSUM"))
        ps_pv = ctx.enter_context(tc.tile_pool(name="ps_pv", bufs=3, space="PSUM"))

        # ---- setup: head-0 weights and xT first so P1(h=0) starts ASAP
        # (it doubles as the PE clock-ramp warmup). Everything split by dc
        # chunk so PE work starts while later DMAs stream in.
        wtiles = {}

        def _alloc_w(h):
            ms = [
                wpool.tile([P, D], dt, tag=f"m{dc}", name=f"m{dc}")
                for dc in range(DC)
            ]
            ns = [
                wpool.tile([P, D], dt, tag=f"n{dc}", name=f"n{dc}")
                for dc in range(DC)
            ]
            wtiles[h] = (ms, ns)

        def _dma_w(h, kind, dc):
            src = m_d if kind == 0 else n_d
            nc.sync.dma_start(
                out=wtiles[h][kind][dc],
                in_=src.ap()[h, dc * P : (dc + 1) * P, :],
            )

        def _fetch_w(h):
            _alloc_w(h)
            for kind in range(2):
                for dc in range(DC):
                    _dma_w(h, kind, dc)

        xts = [[None] * DC for _ in range(2)]

        def _dma_x(sh, dc):
            t_ = consts.tile([P, 512], dt, tag=f"xT{sh}{dc}")
            nc.sync.dma_start(
                out=t_,
                in_=xT_d.ap()[dc * P : (dc + 1) * P, sh * 512 : (sh + 1) * 512],
            )
            xts[sh][dc] = t_

        # prologue in exact first-use order: P1(h0,sh0) needs (m_dc, x0_dc)
        # pairs, then x1 for sh1, then n(h0) for P2, then head-1 weights.
        _alloc_w(0)
        for dc in range(DC):
            _dma_w(0, 0, dc)
            _dma_x(0, dc)
        for dc in range(DC):
            _dma_x(1, dc)
        for dc in range(DC):
            _dma_w(0, 1, dc)
        _fetch_w(1)

        kb_sb = consts.tile([P, NQT], f32, tag="kbT")
        nc.sync.dma_start(out=kb_sb, in_=kb_d.ap())

        maskq_sb = consts.tile([P, NQT], f32, tag="maskq")
        nc.sync.dma_start(
            out=maskq_sb, in_=maskq_d.ap().rearrange("(q p) -> p q", p=P)
        )

        bo_sb = consts.tile([P, D], f32, tag="bo")
        nc.sync.dma_start(out=bo_sb, in_=bo_d.ap())

        # causal min-mask patterns for diagonal chunks: [P, GW] f32,
        # keep (BIG) where s_local >= t_local + off, else RAWNEG.
        mt = []
        for off in (0, 128):
            t_ = consts.tile([P, GW], f32, tag=f"mt{off}")
            nc.vector.memset(t_, BIG)
            nc.gpsimd.affine_select(
                out=t_, in_=t_,
                compare_op=mybir.AluOpType.is_ge,
                fill=RAWNEG, base=-off, channel_multiplier=-1,
                pattern=[[1, GW]],
            )
            mt.append(t_)

        # out accumulator, init = bo * maskq (bias folds done host-side)
        out_acc = consts.tile([P, NQT, D], f32, tag="out_acc")
        for qt in range(NQT):
            nc.vector.tensor_scalar_mul(
                out=out_acc[:, qt, :], in0=bo_sb, scalar1=maskq_sb[:, qt : qt + 1]
            )

        # ---- per-head pipeline ----
        n_heads = int(os.environ.get("MHA_HEADS", str(H)))
        for h in range(n_heads):
            m_t, n_t = wtiles.pop(h)

            # P1: Q'T [e, s]
            qp_sb = qpool.tile([P, DC, S], dt, tag="QT")
            for sh in range(S // 512):
                for ec in range(DC):
                    ps = ps_pj.tile([P, 512], f32, tag="pj")
                    for dc in range(DC):
                        nc.tensor.matmul(
                            ps,
                            m_t[dc][:, ec * P : (ec + 1) * P],
                            xts[sh][dc],
                            start=(dc == 0),
                            stop=(dc == DC - 1),
                        )
                    nc.scalar.copy(
                        out=qp_sb[:, ec, sh * 512 : (sh + 1) * 512], in_=ps
                    )

            # P2: U [t, e] + ones column 384
            u_sb = upool.tile([P, NQT, DU], dt, tag="U")
            nc.vector.memset(u_sb[:, :, D], 1.0)
            for tt in range(NQT):
                psu = ps_pv.tile([P, DU], f32, tag="pv", name="ps_u")
                for dc in range(DC):
                    nc.tensor.matmul(
                        psu[:, :D],
                        xts[tt // 4][dc][:, (tt % 4) * P : (tt % 4 + 1) * P],
                        n_t[dc],
                        start=(dc == 0),
                        stop=(dc == DC - 1),
                    )
                nc.vector.tensor_copy(out=u_sb[:, tt, :D], in_=psu[:, :D])

            # prefetch weights two heads ahead (m/n last read in P1/P2 above)
            if h + 2 < n_heads:
                _fetch_w(h + 2)

            # attention per 256-wide s-group
            for qg in range(NG):
                ntt = 2 * qg + 2          # live t-chunks for this group
                s0 = qg * GW
                att_t = apool.tile([P, NQT, GW], dt, tag="attnT", name="att_t")
                for tt in range(ntt):
                    ps_s = ps_sc.tile([P, GW], f32, tag="sc")
                    for ec in range(DC):
                        nc.tensor.matmul(
                            ps_s,
                            xts[tt // 4][ec][:, (tt % 4) * P : (tt % 4 + 1) * P],
                            qp_sb[:, ec, s0 : s0 + GW],
                            start=(ec == 0),
                            stop=(ec == DC - 1),
                        )
                    if tt >= 2 * qg:  # diagonal chunk: causal min pre-exp
                        nc.vector.tensor_tensor(
                            out=ps_s, in0=ps_s, in1=mt[tt - 2 * qg],
                            op=mybir.AluOpType.min,
                        )
                    nc.scalar.activation(
                        out=att_t[:, tt, :],
                        in_=ps_s,
                        func=mybir.ActivationFunctionType.Exp,
                        scale=INV_SQRT_D,
                        bias=kb_sb[:, tt : tt + 1],
                    )

                # P4 for this group's 2 q-tiles; colsum rides in column 384
                for qi in range(2):
                    qt = 2 * qg + qi
                    ps_p = ps_pv.tile([P, DU], f32, tag="pv")
                    for tt in range(qt + 1):
                        nc.tensor.matmul(
                            ps_p,
                            att_t[:, tt, qi * P : (qi + 1) * P],
                            u_sb[:, tt, :],
                            start=(tt == 0),
                            stop=(tt == qt),
                        )
                    guard = small.tile([P, 1], f32, tag="guard")
                    nc.vector.tensor_scalar_add(
                        out=guard, in0=ps_p[:, D : D + 1], scalar1=1e-30
                    )
                    recip = small.tile([P, 1], f32, tag="recip")
                    nc.vector.reciprocal(out=recip, in_=guard)
                    recipm = small.tile([P, 1], f32, tag="recipm")
                    nc.vector.tensor_tensor(
                        out=recipm, in0=recip,
                        in1=maskq_sb[:, qt : qt + 1],
                        op=mybir.AluOpType.mult,
                    )
                    nc.vector.scalar_tensor_tensor(
                        out=out_acc[:, qt, :],
                        in0=ps_p[:, :D],
                        scalar=recipm,
                        in1=out_acc[:, qt, :],
                        op0=mybir.AluOpType.mult,
                        op1=mybir.AluOpType.add,
                    )

        # ---- final store (maskq and bo already folded into out_acc) ----
        for qt in range(NQT):
            nc.sync.dma_start(
                out=out_d.ap()[qt * P : (qt + 1) * P, :], in_=out_acc[:, qt, :]
            )

    nc.compile()
    return nc


def _in_maps(x, mask, Wq, bq, Wk, bk, Wv, bv, Wo, bo, cfg):
    np_dt = _np_dt(cfg["dt"])
    f32 = np.float32
    x = np.asarray(x, f32)
    Wq = np.asarray(Wq, f32)
    Wk = np.asarray(Wk, f32)
    Wv = np.asarray(Wv, f32)
    Wo = np.asarray(Wo, f32).reshape(H, D, D)
    bq = np.asarray(bq, f32)
    bk = np.asarray(bk, f32)
    bv = np.asarray(bv, f32)
    bo = np.asarray(bo, f32)

    # host precompute: M = Wq Wk^T, N = Wv Wo  (fp32)
    M = np.einsum("hde,hfe->hdf", Wq, Wk)
    N = np.einsum("hde,hef->hdf", Wv, Wo)

    # bias folds (all-zero biases in this problem, kept for generality):
    #   scores += bq.K_t (per-key) -> raw bias columns; Q.bk const/row -> cancels
    #   out += sum_h (bv_h @ Wo_h) + bo  (attn rows sum to 1)
    bo_f = bo + np.einsum("hd,hdf->f", bv, Wo)

    m = np.asarray(mask) != 0
    maskq = m.astype(f32)

    shared = {
        "M": M.astype(np_dt),
        "N": N.astype(np_dt),
        "bo": np.broadcast_to(bo_f[None, :], (P, D)).copy(),
    }
    xT = np.ascontiguousarray(x.transpose(0, 2, 1))  # [B, D, S]
    maps = []
    for b in range(B):
        # per-key exp bias: 0 valid / KNEG masked; plus bq.K_t fold (zero here)
        kb = np.where(m[b], 0.0, np.float32(KNEG)).astype(f32)
        maps.append(
            {
                "xT": xT[b].astype(np_dt),
                "kbT": np.ascontiguousarray(kb.reshape(NQT, P).T),
                "maskq": maskq[b],
                **shared,
            }
        )
    return maps


def run(inputs, trace=False, cfg=None):
    """inputs: dict from setup_inputs(). Returns (out [B,S,D] f32, results)."""
    from concourse.bass_utils import run_bass_kernel_spmd

    global _BUILT
    cfg = dict(CFG if cfg is None else cfg)
    if _BUILT is None or _BUILT[1] != cfg:
        _BUILT = (build(cfg), cfg)
    nc = _BUILT[0]
    in_maps = _in_maps(**inputs, cfg=cfg)
    res = run_bass_kernel_spmd(
        nc, in_maps, core_ids=list(range(B)), trace=trace
    )
    out = np.stack([np.asarray(res.results[b]["out"], np.float32) for b in range(B)])
    return out, res


def kernel(**inputs):
    out, _ = run(inputs, trace=False)
    return out


# revision 28
# speedup vs baseline: 1.2386x; 1.0223x over previous
"""Multi-head self-attention TRN2 kernel (data-parallel over batch).

Problem: B=8, S=1024, D=384, H=8, per-head full D->D projections,
causal + key-padding mask, softmax, out_linear (H*D)->D, query-mask output.

Sharding: batch b -> NeuronCore b (8 cores, no collectives).

Algebraic restructure (host precompute, exact):
  M_h = Wq_h @ Wk_h^T   ->  scores_raw = x M_h x^T   (K-projection eliminated)
  N_h = Wv_h @ Wo_h     ->  out = sum_h softmax(scores) @ (x N_h)  (out-proj eliminated)
  bias folds: Q.bk term is constant per query row -> cancels in softmax;
  bq.K term -> per-key exp bias column; (attn@bv)Wo = bv@Wo (softmax rows sum
  to 1) -> folded into bo on host. All biases are zero in this problem anyway.

Per-core dataflow (one batch element), transpose-free, all bf16 matmuls:
  xT [D,S] bf16 resident in SBUF, split [sh][dc] for DMA/compute pipelining.
  For each head h:
    P1: Q'T[e,s] = M-chunks @ xT          (PE, psum [128,512], -> bf16 SBUF)
    P2: U[t,e|1] = xT-chunks @ N, col 384 = ones  (PE, -> bf16 SBUF)
    per 256-wide q group (4 groups, causally-live t-chunks only):
      P3: scoresT[t,s] psum = xT-chunk stationary @ Q'T   (raw, unscaled)
      diagonal chunks: min(scores, MTpat) in-psum (DVE), 2 const patterns
      attnT[t,s] = exp(scores*inv_sqrt_d + keybias[t]) -> bf16 SBUF (ACT)
      P4 per q-tile (128): psum[s, 0:385] = sum_t attnT-chunk stat @ U
        -> psum[:,384] is the colsum; recip'[s] = maskq/(colsum+eps) (DVE)
        -> out_acc[s,:] += psum[:, :384] * recip'[s]    (DVE STT)
  out = out_acc (maskq,bo pre-folded) -> DRAM
"""

import os
from contextlib import ExitStack

import numpy as np

B, S, D, H = 8, 1024, 384, 8
P = 128
DC = D // P          # 3 partition chunks of the d/e axes
NQT = S // P         # 8 q/t tiles of 128
GW = 256             # q-group width for scores/exp
NG = S // GW         # 4 groups
DU = D + 1           # U width incl. ones column for in-P4 colsum
BIG = 3.0e38
INV_SQRT_D = float(1.0 / np.sqrt(np.float32(D), dtype=np.float32))
KNEG = -120.0                      # exp bias for masked keys -> exp==0 in bf16
RAWNEG = float(KNEG / INV_SQRT_D)  # raw-score causal fill; scaled -> -120

CFG = {"dt": os.environ.get("MHA_DT", "bf16")}

_BUILT = None  # (nc, cfg)


def _dt(kind):
    import concourse.mybir as mybir

    return {"bf16": mybir.dt.bfloat16, "f32r": mybir.dt.float32r,
            "f32": mybir.dt.float32}[kind]


def _np_dt(kind):
    import ml_dtypes

    return ml_dtypes.bfloat16 if kind == "bf16" else np.float32


def build(cfg=None):
    import concourse.bass as bass
    import concourse.bacc as bacc
    import concourse.tile as tile
    import concourse.mybir as mybir

    cfg = dict(CFG if cfg is None else cfg)
    f32 = mybir.dt.float32
    dt = _dt(cfg["dt"])

    nc = bacc.Bacc("TRN2", target_bir_lowering=False, debug=False)

    xT_d = nc.dram_tensor("xT", [D, S], dt, kind="ExternalInput")
    m_d = nc.dram_tensor("M", [H, D, D], dt, kind="ExternalInput")
    n_d = nc.dram_tensor("N", [H, D, D], dt, kind="ExternalInput")
    kb_d = nc.dram_tensor("kbT", [P, NQT], f32, kind="ExternalInput")
    maskq_d = nc.dram_tensor("maskq", [S], f32, kind="ExternalInput")
    bo_d = nc.dram_tensor("bo", [P, D], f32, kind="ExternalInput")
    out_d = nc.dram_tensor("out", [S, D], f32, kind="ExternalOutput")

    with tile.TileContext(nc) as tc, ExitStack() as ctx:
        consts = ctx.enter_context(tc.tile_pool(name="consts", bufs=1))
        wpool = ctx.enter_context(tc.tile_pool(name="wpool", bufs=2))
        qpool = ctx.enter_context(tc.tile_pool(name="qpool", bufs=2))
        upool = ctx.enter_context(tc.tile_pool(name="upool", bufs=2))
        apool = ctx.enter_context(tc.tile_pool(name="apool", bufs=3))
        small = ctx.enter_context(tc.tile_pool(name="small", bufs=8))
        ps_pj = ctx.enter_context(tc.tile_pool(name="ps_pj", bufs=2, space="PSUM"))
        ps_sc = ctx.enter_context(tc.tile_pool(name="ps_sc", bufs=3, space="PSUM"))
        ps_pv = ctx.enter_context(tc.tile_pool(name="ps_pv", bufs=3, space="PSUM"))

        # ---- setup: head-0 weights and xT first so P1(h=0) starts ASAP
        # (it doubles as the PE clock-ramp warmup). Everything split by dc
        # chunk so PE work starts while later DMAs stream in.
        wtiles = {}

        def _alloc_w(h):
            ms = [
                wpool.tile([P, D], dt, tag=f"m{dc}", name=f"m{dc}")
                for dc in range(DC)
            ]
            ns = [
                wpool.tile([P, D], dt, tag=f"n{dc}", name=f"n{dc}")
                for dc in range(DC)
            ]
            wtiles[h] = (ms, ns)

        def _dma_w(h, kind, dc):
            src = m_d if kind == 0 else n_d
            nc.sync.dma_start(
                out=wtiles[h][kind][dc],
                in_=src.ap()[h, dc * P : (dc + 1) * P, :],
            )

        def _fetch_w(h):
            _alloc_w(h)
            for kind in range(2):
                for dc in range(DC):
                    _dma_w(h, kind, dc)

        xts = [[None] * DC for _ in range(2)]

        def _dma_x(sh, dc):
            # issue on the Activation HWDGE queue: streams in parallel with
            # the weight DMAs on the Sync queue during the prologue
            t_ = consts.tile([P, 512], dt, tag=f"xT{sh}{dc}")
            nc.scalar.dma_start(
                out=t_,
                in_=xT_d.ap()[dc * P : (dc + 1) * P, sh * 512 : (sh + 1) * 512],
            )
            xts[sh][dc] = t_

        # prologue in exact first-use order: P1(h0,sh0) needs (m_dc, x0_dc)
        # pairs, then x1 for sh1, then n(h0) for P2, then head-1 weights.
        _alloc_w(0)
        for dc in range(DC):
            _dma_w(0, 0, dc)
            _dma_x(0, dc)
        for dc in range(DC):
            _dma_x(1, dc)
        for dc in range(DC):
            _dma_w(0, 1, dc)
        _fetch_w(1)

        kb_sb = consts.tile([P, NQT], f32, tag="kbT")
        nc.sync.dma_start(out=kb_sb, in_=kb_d.ap())

        maskq_sb = consts.tile([P, NQT], f32, tag="maskq")
        nc.sync.dma_start(
            out=maskq_sb, in_=maskq_d.ap().rearrange("(q p) -> p q", p=P)
        )

        bo_sb = consts.tile([P, D], f32, tag="bo")
        nc.sync.dma_start(out=bo_sb, in_=bo_d.ap())

        # causal min-mask patterns for diagonal chunks: [P, GW] f32,
        # keep (BIG) where s_local >= t_local + off, else RAWNEG.
        mt = []
        for off in (0, 128):
            t_ = consts.tile([P, GW], f32, tag=f"mt{off}")
            nc.vector.memset(t_, BIG)
            nc.gpsimd.affine_select(
                out=t_, in_=t_,
                compare_op=mybir.AluOpType.is_ge,
                fill=RAWNEG, base=-off, channel_multiplier=-1,
                pattern=[[1, GW]],
            )
            mt.append(t_)

        # out accumulator, init = bo * maskq (bias folds done host-side)
        out_acc = consts.tile([P, NQT, D], f32, tag="out_acc")
        for qt in range(NQT):
            nc.vector.tensor_scalar_mul(
                out=out_acc[:, qt, :], in0=bo_sb, scalar1=maskq_sb[:, qt : qt + 1]
            )

        # ---- per-head pipeline ----
        n_heads = int(os.environ.get("MHA_HEADS", str(H)))
        for h in range(n_heads):
            m_t, n_t = wtiles.pop(h)

            # P1: Q'T [e, s]
            qp_sb = qpool.tile([P, DC, S], dt, tag="QT")
            for sh in range(S // 512):
                for ec in range(DC):
                    ps = ps_pj.tile([P, 512], f32, tag="pj")
                    for dc in range(DC):
                        nc.tensor.matmul(
                            ps,
                            m_t[dc][:, ec * P : (ec + 1) * P],
                            xts[sh][dc],
                            start=(dc == 0),
                            stop=(dc == DC - 1),
                        )
                    nc.scalar.copy(
                        out=qp_sb[:, ec, sh * 512 : (sh + 1) * 512], in_=ps
                    )

            # P2: U [t, e] + ones column 384
            u_sb = upool.tile([P, NQT, DU], dt, tag="U")
            nc.vector.memset(u_sb[:, :, D], 1.0)
            for tt in range(NQT):
                psu = ps_pv.tile([P, DU], f32, tag="pv", name="ps_u")
                for dc in range(DC):
                    nc.tensor.matmul(
                        psu[:, :D],
                        xts[tt // 4][dc][:, (tt % 4) * P : (tt % 4 + 1) * P],
                        n_t[dc],
                        start=(dc == 0),
                        stop=(dc == DC - 1),
                    )
                nc.vector.tensor_copy(out=u_sb[:, tt, :D], in_=psu[:, :D])

            # prefetch weights two heads ahead (m/n last read in P1/P2 above)
            if h + 2 < n_heads:
                _fetch_w(h + 2)

            # attention per 256-wide s-group
            for qg in range(NG):
                ntt = 2 * qg + 2          # live t-chunks for this group
                s0 = qg * GW
                att_t = apool.tile([P, NQT, GW], dt, tag="attnT", name="att_t")
                for tt in range(ntt):
                    ps_s = ps_sc.tile([P, GW], f32, tag="sc")
                    for ec in range(DC):
                        nc.tensor.matmul(
                            ps_s,
                            xts[tt // 4][ec][:, (tt % 4) * P : (tt % 4 + 1) * P],
                            qp_sb[:, ec, s0 : s0 + GW],
                            start=(ec == 0),
                            stop=(ec == DC - 1),
                        )
                    if tt >= 2 * qg:  # diagonal chunk: causal min pre-exp
                        nc.vector.tensor_tensor(
                            out=ps_s, in0=ps_s, in1=mt[tt - 2 * qg],
                            op=mybir.AluOpType.min,
                        )
                    nc.scalar.activation(
                        out=att_t[:, tt, :],
                        in_=ps_s,
                        func=mybir.ActivationFunctionType.Exp,
                        scale=INV_SQRT_D,
                        bias=kb_sb[:, tt : tt + 1],
                    )

                # P4 for this group's 2 q-tiles; colsum rides in column 384
                for qi in range(2):
                    qt = 2 * qg + qi
                    ps_p = ps_pv.tile([P, DU], f32, tag="pv")
                    for tt in range(qt + 1):
                        nc.tensor.matmul(
                            ps_p,
                            att_t[:, tt, qi * P : (qi + 1) * P],
                            u_sb[:, tt, :],
                            start=(tt == 0),
                            stop=(tt == qt),
                        )
                    guard = small.tile([P, 1], f32, tag="guard")
                    nc.vector.tensor_scalar_add(
                        out=guard, in0=ps_p[:, D : D + 1], scalar1=1e-30
                    )
                    recip = small.tile([P, 1], f32, tag="recip")
                    nc.vector.reciprocal(out=recip, in_=guard)
                    recipm = small.tile([P, 1], f32, tag="recipm")
                    nc.vector.tensor_tensor(
                        out=recipm, in0=recip,
                        in1=maskq_sb[:, qt : qt + 1],
                        op=mybir.AluOpType.mult,
                    )
                    nc.vector.scalar_tensor_tensor(
                        out=out_acc[:, qt, :],
                        in0=ps_p[:, :D],
                        scalar=recipm,
                        in1=out_acc[:, qt, :],
                        op0=mybir.AluOpType.mult,
                        op1=mybir.AluOpType.add,
                    )

        # ---- final store (maskq and bo already folded into out_acc) ----
        for qt in range(NQT):
            nc.sync.dma_start(
                out=out_d.ap()[qt * P : (qt + 1) * P, :], in_=out_acc[:, qt, :]
            )

    nc.compile()
    return nc


def _in_maps(x, mask, Wq, bq, Wk, bk, Wv, bv, Wo, bo, cfg):
    np_dt = _np_dt(cfg["dt"])
    f32 = np.float32
    x = np.asarray(x, f32)
    Wq = np.asarray(Wq, f32)
    Wk = np.asarray(Wk, f32)
    Wv = np.asarray(Wv, f32)
    Wo = np.asarray(Wo, f32).reshape(H, D, D)
    bq = np.asarray(bq, f32)
    bk = np.asarray(bk, f32)
    bv = np.asarray(bv, f32)
    bo = np.asarray(bo, f32)

    # host precompute: M = Wq Wk^T, N = Wv Wo  (fp32)
    M = np.einsum("hde,hfe->hdf", Wq, Wk)
    N = np.einsum("hde,hef->hdf", Wv, Wo)

    # bias folds (all-zero biases in this problem, kept for generality):
    #   scores += bq.K_t (per-key) -> raw bias columns; Q.bk const/row -> cancels
    #   out += sum_h (bv_h @ Wo_h) + bo  (attn rows sum to 1)
    bo_f = bo + np.einsum("hd,hdf->f", bv, Wo)

    m = np.asarray(mask) != 0
    maskq = m.astype(f32)

    shared = {
        "M": M.astype(np_dt),
        "N": N.astype(np_dt),
        "bo": np.broadcast_to(bo_f[None, :], (P, D)).copy(),
    }
    xT = np.ascontiguousarray(x.transpose(0, 2, 1))  # [B, D, S]
    maps = []
    for b in range(B):
        # per-key exp bias: 0 valid / KNEG masked; plus bq.K_t fold (zero here)
        kb = np.where(m[b], 0.0, np.float32(KNEG)).astype(f32)
        maps.append(
            {
                "xT": xT[b].astype(np_dt),
                "kbT": np.ascontiguousarray(kb.reshape(NQT, P).T),
                "maskq": maskq[b],
                **shared,
            }
        )
    return maps


def run(inputs, trace=False, cfg=None):
    """inputs: dict from setup_inputs(). Returns (out [B,S,D] f32, results)."""
    from concourse.bass_utils import run_bass_kernel_spmd

    global _BUILT
    cfg = dict(CFG if cfg is None else cfg)
    if _BUILT is None or _BUILT[1] != cfg:
        _BUILT = (build(cfg), cfg)
    nc = _BUILT[0]
    in_maps = _in_maps(**inputs, cfg=cfg)
    res = run_bass_kernel_spmd(
        nc, in_maps, core_ids=list(range(B)), trace=trace
    )
    out = np.stack([np.asarray(res.results[b]["out"], np.float32) for b in range(B)])
    return out, res


def kernel(**inputs):
    out, _ = run(inputs, trace=False)
    return out
